# revision 22
# baseline (speedup 1.0000x reference)
"""DCNv2 (modulated deformable convolution) on 8 Trainium2 NeuronCores.

kernel(**inputs) takes the full unsharded inputs
    x      (8, 128, 64, 64) f32
    w_om   (27, 128, 3, 3)  f32
    b_om   (27,)            f32
    weight (128, 128, 3, 3) f32
    bias   (128,)           f32
and returns the full output (8, 128, 64, 64) f32.

Sharding: pure data-parallel over batch — one image per NeuronCore, small
weights replicated; no collectives.

Per-core Bass/Tile program (v4):
  1. offset conv (27ch 3x3) on the PE as 9 shifted bf16 matmuls over padded x
  2. softmax mask + bilinear coefficients + gather indices on DVE/ACT in a
     (pixel-partition, (tile,tap)-free) layout; the wrapped-16 index layout
     dma_gather wants is produced with exact identity-slice matmuls on the
     PE (coords <= 70 stay exact in reduced precision) + strided DVE casts
  3. a 4-corner-duplicated padded image Z (72*72 rows x 4x128ch bf16) is
     staged in DRAM: Z[y,x] = [x(y,x), x(y,x+1), x(y+1,x), x(y+1,x+1)], so a
     single 1KB gather descriptor fetches all 4 bilinear corners of one
     (pixel, tap) sample
  4. per (half, tap): one dma_gather, 7 wide DVE ops (bf16, coefficient
     broadcast along the free dim) combine the corners, 16 PE transposes
     back to (channel, pixel) and 4 accumulating bf16 matmuls with the
     128x128x3x3 weight; bias is added on the PSUM->SBUF copy.

The whole offset->coefficient->index chain is split per image half
(independent tiles) and PE work is emitted in dependency-priority order so
the first gather launches as early as possible; gpsimd descriptor
generation (~16.5us per 2048-index gather) is the pacing resource.
"""

import os
import sys

import numpy as np

sys.path.insert(0, "/opt/trn_rl_repo")

from contextlib import ExitStack

import concourse.bacc as bacc
import concourse.mybir as mybir
import concourse.tile as tile
from concourse._compat import get_trn_type
from concourse.alu_op_type import AluOpType as Alu
from concourse.bass import AP
from concourse.bass_utils import run_bass_kernel_spmd
from concourse import library_config

F32 = mybir.dt.float32
BF16 = mybir.dt.bfloat16
I32 = mybir.dt.int32
I16 = mybir.dt.int16

B = 8
C = 128
H = W = 64
HW = H * W
K2 = 9
PADG = 4
GW = H + 2 * PADG      # 72
GROWS = GW * GW        # 5184
NS = 32
NHALF = 2
SH = NS // NHALF             # 16 s-tiles per half
PIX_PER_HALF = HW // NHALF   # 2048

LAST_EXEC_TIME_NS = None


def _emit(tc):
    nc = tc.nc
    x_d = nc.dram_tensor("x", [C, HW], F32, kind="ExternalInput").ap()
    w_om_d = nc.dram_tensor("w_om", [27, 1152], F32, kind="ExternalInput").ap()
    b_om_d = nc.dram_tensor("b_om", [27, 1], F32, kind="ExternalInput").ap()
    weight_d = nc.dram_tensor("weight", [C, 1152], F32, kind="ExternalInput").ap()
    bias_d = nc.dram_tensor("bias", [C, 1], F32, kind="ExternalInput").ap()
    out_d = nc.dram_tensor("out", [C, HW], F32, kind="ExternalOutput").ap()
    z_d = nc.dram_tensor("z_pad", [GROWS, 512], BF16, kind="ExternalOutput").ap()
    consts_d = nc.dram_tensor("consts", [128, 707], F32, kind="ExternalInput").ap()

    nc.gpsimd.load_library(library_config.mlp)

    ctx = ExitStack()
    with ctx:
        cpool = ctx.enter_context(tc.tile_pool(name="const", bufs=1))
        spool = ctx.enter_context(tc.tile_pool(name="setup", bufs=1))
        dpool = ctx.enter_context(tc.tile_pool(name="data", bufs=1))
        gpool = ctx.enter_context(tc.tile_pool(name="gath", bufs=3))
        vpool = ctx.enter_context(tc.tile_pool(name="val", bufs=2))
        ppool = ctx.enter_context(tc.tile_pool(name="psum", bufs=1, space="PSUM"))
        tpool = ctx.enter_context(tc.tile_pool(name="trps", bufs=2, space="PSUM"))
        opool = ctx.enter_context(tc.tile_pool(name="omps", bufs=2, space="PSUM"))

        # ---------- constants ----------
        cons = cpool.tile([128, 707], F32)
        nc.sync.dma_start(cons[:], consts_d[:, :])
        ident = cons[:, 0:128]
        hob = cons[:, 129:130]
        wo_r = cons[:, 130:131]

        ident_bf = spool.tile([128, 128], BF16)
        nc.vector.tensor_copy(ident_bf[:], ident)

        # ---------- load x; bf16 cast; x_pad ----------
        x_sb = dpool.tile([128, HW], F32, tag="big16k")
        for q in range(4):
            nc.sync.dma_start(x_sb[:, q * 1024:(q + 1) * 1024],
                              x_d[:, q * 1024:(q + 1) * 1024])
        x_bf = spool.tile([128, HW], BF16)
        for q in range(4):
            nc.vector.tensor_copy(x_bf[:, q * 1024:(q + 1) * 1024],
                                  x_sb[:, q * 1024:(q + 1) * 1024])

        XP = 66
        x_pad = spool.tile([128, XP * XP], BF16)
        nc.vector.memset(x_pad[:], 0.0)

        # Z border zero-fill (no deps beyond the memset; fire early)
        zero_bf = spool.tile([128, 1152], BF16)
        nc.vector.memset(zero_bf[:], 0.0)
        nc.sync.dma_start(
            AP(z_d.tensor, 0, [[1152, 128], [1, 1152]]), zero_bf[:])
        nc.sync.dma_start(
            AP(z_d.tensor, (GW - PADG) * GW * 512, [[1152, 128], [1, 1152]]),
            zero_bf[:])
        nc.sync.dma_start(
            AP(z_d.tensor, PADG * GW * 512, [[GW * 512, 64], [1, 2048]]),
            zero_bf[:, 0:1024])
        nc.sync.dma_start(
            AP(z_d.tensor, (PADG * GW + GW - PADG) * 512,
               [[GW * 512, 64], [1, 2048]]),
            zero_bf[:, 0:1024])

        stg_all = spool.tile([128, NS, 128], BF16)

        # ---------- Z interior (before the om chain: the PE transposes can
        # start as soon as the x quarters land, and the corner DMAs use the
        # otherwise-idle early sync window) ----------
        for s in range(NS):
            trp = tpool.tile([128, 128], BF16, tag="trb", name="trp")
            nc.tensor.transpose(trp[:], x_bf[:, s * 128:(s + 1) * 128],
                                ident_bf)
            nc.scalar.copy(stg_all[:, s, :], trp[:])
        for q in range(4):
            nc.sync.dma_start(
                x_pad[:].rearrange("p (a b) -> p a b", a=XP)
                [:, 1 + 16 * q:1 + 16 * (q + 1), 1:65],
                x_bf[:, q * 1024:(q + 1) * 1024])
        for cc, (ry, rx) in enumerate(((0, 0), (0, 1), (1, 0), (1, 1))):
            for r in range(2):
                dst = AP(
                    z_d.tensor,
                    ((r + PADG - ry) * GW + PADG - rx) * 512 + cc * 128,
                    [[512, 64], [2 * GW * 512, NS], [1, 128]],
                )
                nc.sync.dma_start(dst, stg_all[64 * r:64 * r + 64, :, :])

        # ---------- weights ----------
        w_om_sb = spool.tile([27, 1152], F32)
        nc.sync.dma_start(w_om_sb[:], w_om_d[:, :])
        womb = spool.tile([27, 1152], BF16)
        nc.vector.tensor_copy(womb[:], w_om_sb[:])
        b_om_sb = spool.tile([27, 1], F32)
        nc.sync.dma_start(b_om_sb[:], b_om_d[:, :])
        weight_sb = spool.tile([128, 1152], F32)
        nc.sync.dma_start(weight_sb[:], weight_d[:, :])
        wbf = spool.tile([128, 1152], BF16)
        nc.vector.tensor_copy(wbf[:], weight_sb[:])
        bias_sb = spool.tile([128, 1], F32)
        nc.sync.dma_start(bias_sb[:], bias_d[:, :])


        # womT first: it gates the offset conv (the longest setup chain)
        womT = spool.tile([128, K2, 27], BF16)
        for k in range(K2):
            trp = tpool.tile([128, 128], BF16, tag="trb", name="trp")
            nc.tensor.transpose(
                trp[:, :27], womb[:].rearrange("p (c k) -> p c k", k=K2)[:, :, k],
                ident_bf[0:27, 0:27],
            )
            nc.scalar.copy(womT[:, k, :], trp[:, :27])

        xpv = x_pad[:].rearrange("p (a b) -> p a b", a=XP)

        _cnt = [0]

        def f(shape=(128, SH, K2), dt=F32, tag=None):
            _cnt[0] += 1
            nm = f"cf{_cnt[0]}"
            return dpool.tile(list(shape), dt, tag=tag or nm, name=nm)

        def floorit(v):
            vi = f(dt=I32, tag="fl_i")
            nc.vector.tensor_copy(vi[:], v[:])
            v0 = f(tag="fl_f")
            nc.vector.tensor_copy(v0[:], vi[:])
            gt = f(tag="fl_gt")
            nc.vector.tensor_tensor(gt[:], v0[:], v[:], Alu.is_gt)
            v0f = f()
            nc.vector.tensor_tensor(v0f[:], v0[:], gt[:], Alu.subtract)
            return v0f

        idxw = dpool.tile([128, K2 * 256], I16)
        idxw_t = idxw[:].tensor
        idxw_off = idxw[:].offset

        cbs = []  # per half: (cb00, cb01, cb10, cb11)
        wT = spool.tile([128, K2, 128], BF16)

        def emit_z_and_wt():
            # main-conv weight transposes (Z itself is built early, above)
            for k in range(K2):
                trp = tpool.tile([128, 128], BF16, tag="trb", name="trp")
                nc.tensor.transpose(
                    trp[:], wbf[:].rearrange("p (c k) -> p c k", k=K2)[:, :, k],
                    ident_bf,
                )
                nc.scalar.copy(wT[:, k, :], trp[:])

        for hh in range(NHALF):
            # ---- offset conv for rows [32*hh, 32*hh+32): om in bf16 ----
            om_bf = spool.tile([27, HW // 2], BF16, name=f"om{hh}")
            for chl in range(4):
                ch = 4 * hh + chl
                omp = opool.tile([27, 512], F32, tag="om", name="omp")
                for k in range(K2):
                    dy_, dx_ = k // 3, k % 3
                    r0 = ch * 8 + dy_
                    nc.tensor.matmul(
                        omp[:], womT[:, k, :27], xpv[:, r0:r0 + 8, dx_:dx_ + 64],
                        start=(k == 0), stop=(k == K2 - 1),
                    )
                nc.scalar.activation(
                    om_bf[:, chl * 512:(chl + 1) * 512], omp[:],
                    mybir.ActivationFunctionType.Identity, bias=b_om_sb[:],
                    scale=1.0,
                )

            # ---- omT (128 pix, 27) per s-tile ----
            omT = spool.tile([128, SH, 27], BF16, name=f"omT{hh}")
            for s in range(SH):
                trp = tpool.tile([128, 128], BF16, tag="trb", name="trp")
                nc.tensor.transpose(
                    trp[:, :27], om_bf[:, s * 128:(s + 1) * 128],
                    ident_bf[0:27, 0:27],
                )
                nc.scalar.copy(omT[:, s, :], trp[:, :27])

            # ---- sampling positions (padded coords; tables carry +PADG) ----
            omT_t = omT[:].tensor
            omT_off = omT[:].offset
            dyT = AP(omT_t, omT_off + 0, [[SH * 27, 128], [27, SH], [2, K2]])
            dxT = AP(omT_t, omT_off + 1, [[SH * 27, 128], [27, SH], [2, K2]])
            mlg = omT[:, :, 18:27]
            ykv = cons[:, 131 + 144 * hh:131 + 144 * hh + 144].rearrange(
                "p (s a) -> p s a", a=K2)
            xkv = cons[:, 419 + 144 * hh:419 + 144 * hh + 144].rearrange(
                "p (s a) -> p s a", a=K2)

            py = f()
            nc.vector.scalar_tensor_tensor(py[:], dyT, hob, ykv, Alu.add, Alu.add)
            px = f()
            nc.vector.scalar_tensor_tensor(px[:], dxT, wo_r, xkv, Alu.add, Alu.add)
            y0f = floorit(py)
            x0f = floorit(px)

            # clipped integer corner coords in padded Z space: [0, 70]
            yx = dpool.tile([128, 2, SH, K2], F32, name=f"yx{hh}")
            nc.vector.tensor_scalar(yx[:, 0, :, :], y0f[:], 0.0, float(GW - 2),
                                    Alu.max, Alu.min)
            nc.vector.tensor_scalar(yx[:, 1, :, :], x0f[:], 0.0, float(GW - 2),
                                    Alu.max, Alu.min)

            # ---- idx fold to the gather's wrapped-16 layout ----
            # idxw[16g+m, k*256 + 128*hh + 8s + u] = zrow(pixel of
            #   (s-tile 16*hh+s, lane 16u+m), tap k), for all groups g
            stage = dpool.tile([16, 288], F32, name=f"stg{hh}")
            stage2 = dpool.tile([16, 144], F32, name=f"stg2{hh}")
            for u in range(8):
                pyp = opool.tile([27, 512], F32, tag="om", name="omp")
                nc.tensor.matmul(pyp[0:16, 0:144], ident[:, 16 * u:16 * u + 16],
                                 yx[:, 0, :, :], start=True, stop=True)
                pxp = opool.tile([27, 512], F32, tag="om", name="omp")
                nc.tensor.matmul(pxp[0:16, 0:144], ident[:, 16 * u:16 * u + 16],
                                 yx[:, 1, :, :], start=True, stop=True)
                nc.scalar.copy(stage[:, 0:144], pyp[0:16, 0:144])
                nc.scalar.copy(stage[:, 144:288], pxp[0:16, 0:144])
                # zrow = y*GW + x
                nc.vector.scalar_tensor_tensor(
                    stage2[:], stage[:, 0:144], float(GW), stage[:, 144:288],
                    Alu.mult, Alu.add)
                src = AP(stage2[:].tensor, stage2[:].offset,
                         [[144, 16], [9, SH], [1, K2]])
                dst = AP(idxw_t, idxw_off + 1152 * hh + u,
                         [[K2 * 256, 16], [8, SH], [128, K2]])
                nc.vector.tensor_copy(dst, src)
            for lo, hi in ((16, 32), (32, 64), (64, 128)):
                nc.sync.dma_start(
                    idxw[lo:hi, 1152 * hh:1152 * hh + 1152],
                    idxw[0:hi - lo, 1152 * hh:1152 * hh + 1152])

            # ---- mask + bilinear coefficients (deferred: off the gather
            #      critical path, overlapped with the first gathers) ----
            def make_coeffs(hh=hh, py=py, px=px, y0f=y0f, x0f=x0f, mlg=mlg):
                e = f()
                nc.scalar.activation(e[:], mlg,
                                     mybir.ActivationFunctionType.Exp)
                ssum = f((128, SH, 1))
                nc.vector.tensor_reduce(ssum[:], e[:], mybir.AxisListType.X,
                                        Alu.add)
                rs = f((128, SH, 1))
                nc.vector.reciprocal(rs[:], ssum[:])
                mask = f()
                nc.vector.tensor_tensor(mask[:], e[:],
                                        rs[:].to_broadcast([128, SH, K2]),
                                        Alu.mult)
                wy1 = f()
                nc.vector.tensor_tensor(wy1[:], py[:], y0f[:], Alu.subtract)
                wy0 = f()
                nc.vector.tensor_scalar(wy0[:], wy1[:], -1.0, 1.0, Alu.mult,
                                        Alu.add)
                wx1 = f()
                nc.vector.tensor_tensor(wx1[:], px[:], x0f[:], Alu.subtract)
                wx0 = f()
                nc.vector.tensor_scalar(wx0[:], wx1[:], -1.0, 1.0, Alu.mult,
                                        Alu.add)
                mwy0 = f()
                nc.vector.tensor_tensor(mwy0[:], mask[:], wy0[:], Alu.mult)
                mwy1 = f()
                nc.vector.tensor_tensor(mwy1[:], mask[:], wy1[:], Alu.mult)
                # ccat4[p, s, k, corner] bf16 — corner-interleaved so the
                # main loop can apply all 4 corners in one 4D DVE op
                ccat4 = dpool.tile([128, SH, K2, 4], BF16, name=f"cc4_{hh}")
                cc_t = ccat4[:].tensor
                cc_off = ccat4[:].offset
                for ci, (a_, b_) in enumerate(((mwy0, wx0), (mwy0, wx1),
                                               (mwy1, wx0), (mwy1, wx1))):
                    cf = f()
                    nc.vector.tensor_tensor(cf[:], a_[:], b_[:], Alu.mult)
                    dst = AP(cc_t, cc_off + ci,
                             [[SH * K2 * 4, 128], [K2 * 4, SH], [4, K2]])
                    nc.vector.tensor_copy(dst, cf[:])
                return ccat4

            cbs.append(make_coeffs())

            if hh == 0:
                emit_z_and_wt()

        # ---------- Z (4-corner duplicated padded image) in DRAM ----------
        # ---------- main loop ----------
        out_sb = dpool.tile([128, HW], F32, tag="big16k")
        z_src = AP(z_d.tensor, 0, [[512, GROWS], [1, 512]])
        shp = [128, SH, 128]
        for h in range(NHALF):
            cb4 = cbs[h]
            outp = ppool.tile([128, PIX_PER_HALF], F32, tag="out", name="outp")
            for k in range(K2):
                gZ = gpool.tile([128, SH, 512], BF16, tag="gZ", name="gZ")
                nc.gpsimd.dma_gather(
                    gZ[:], z_src,
                    idxw[:, 1152 * h + 128 * k: 1152 * h + 128 * k + 128],
                    PIX_PER_HALF, PIX_PER_HALF, 512, elem_step=512,
                    single_packet=False,
                )

                def cb(ci):
                    s_ = cb4[:, :, k, ci:ci + 1]
                    return s_.to_broadcast(shp)

                a = vpool.tile(shp, BF16, tag="pa", name="pa")
                nc.vector.tensor_tensor(a[:], gZ[:, :, 0:128], cb(0), Alu.mult)
                b = vpool.tile(shp, BF16, tag="pb", name="pb")
                nc.vector.tensor_tensor(b[:], gZ[:, :, 128:256], cb(1), Alu.mult)
                v = vpool.tile(shp, BF16, tag="pv", name="pv")
                nc.vector.tensor_tensor(v[:], a[:], b[:], Alu.add)
                a2 = vpool.tile(shp, BF16, tag="pa", name="pa")
                nc.vector.tensor_tensor(a2[:], gZ[:, :, 256:384], cb(2), Alu.mult)
                b2 = vpool.tile(shp, BF16, tag="pb", name="pb")
                nc.vector.tensor_tensor(b2[:], gZ[:, :, 384:512], cb(3), Alu.mult)
                v2 = vpool.tile(shp, BF16, tag="pv", name="pv")
                nc.vector.tensor_tensor(v2[:], a2[:], b2[:], Alu.add)
                nc.vector.tensor_tensor(v[:], v[:], v2[:], Alu.add)

                vT = vpool.tile([128, 4 * 512], BF16, tag="vT", name="vT")
                for t in range(SH):
                    trp = tpool.tile([128, 128], BF16, tag="trb", name="trp")
                    nc.tensor.transpose(trp[:], v[:, t, :], ident_bf)
                    nc.scalar.copy(vT[:, t * 128:(t + 1) * 128], trp[:])
                for bk in range(4):
                    nc.tensor.matmul(
                        outp[:, bk * 512:(bk + 1) * 512], wT[:, k, :],
                        vT[:, bk * 512:(bk + 1) * 512],
                        start=(k == 0), stop=(k == K2 - 1),
                    )
            for bk in range(4):
                nc.scalar.activation(
                    out_sb[:, h * PIX_PER_HALF + bk * 512:
                           h * PIX_PER_HALF + (bk + 1) * 512],
                    outp[:, bk * 512:(bk + 1) * 512],
                    mybir.ActivationFunctionType.Identity, bias=bias_sb[:],
                    scale=1.0,
                )
        nc.sync.dma_start(out_d[:, :], out_sb[:])


def _make_consts():
    c = np.zeros((128, 707), np.float32)
    c[:, 0:128] = np.eye(128, dtype=np.float32)
    p = np.arange(128)
    c[:, 128] = p
    c[:, 129] = (p >= 64)
    c[:, 130] = p % 64
    s = np.arange(32)[:, None, None]
    kyv = np.arange(3)[None, :, None]
    kxv = np.arange(3)[None, None, :]
    c[:, 131:419] = np.broadcast_to(
        (2 * s + kyv - 1 + PADG + 0 * kxv).reshape(-1), (128, 288))
    c[:, 419:707] = np.broadcast_to(
        (0 * s + 0 * kyv + kxv - 1 + PADG).reshape(-1), (128, 288))
    return c


_COMPILED = None


def _get_compiled():
    global _COMPILED
    if _COMPILED is None:
        nc = bacc.Bacc(get_trn_type() or "TRN2", target_bir_lowering=False,
                       debug=False, num_devices=B)
        with tile.TileContext(nc) as tc:
            _emit(tc)
        nc.compile()
        _COMPILED = nc
    return _COMPILED


def kernel(x, w_om, b_om, weight, bias):
    global LAST_EXEC_TIME_NS
    x = np.ascontiguousarray(np.asarray(x, dtype=np.float32))
    w_om_f = np.ascontiguousarray(np.asarray(w_om, np.float32).reshape(27, 1152))
    b_om_f = np.ascontiguousarray(np.asarray(b_om, np.float32).reshape(27, 1))
    weight_f = np.ascontiguousarray(np.asarray(weight, np.float32).reshape(128, 1152))
    bias_f = np.ascontiguousarray(np.asarray(bias, np.float32).reshape(128, 1))

    nc = _get_compiled()
    consts = _make_consts()
    in_maps = [
        {
            "x": np.ascontiguousarray(x[b].reshape(C, HW)),
            "w_om": w_om_f,
            "b_om": b_om_f,
            "weight": weight_f,
            "bias": bias_f,
            "consts": consts,
        }
        for b in range(B)
    ]
    trace = bool(os.environ.get("DCN_TRACE"))
    res = run_bass_kernel_spmd(nc, in_maps, core_ids=list(range(B)), trace=trace)
    LAST_EXEC_TIME_NS = res.exec_time_ns
    out = np.stack([res.results[b]["out"].reshape(C, H, W) for b in range(B)])
    return out.astype(np.float32)


# revision 23
# speedup vs baseline: 1.0992x; 1.0992x over previous
"""DCNv2 (modulated deformable convolution) on 8 Trainium2 NeuronCores.

kernel(**inputs) takes the full unsharded inputs
    x      (8, 128, 64, 64) f32
    w_om   (27, 128, 3, 3)  f32
    b_om   (27,)            f32
    weight (128, 128, 3, 3) f32
    bias   (128,)           f32
and returns the full output (8, 128, 64, 64) f32.

Sharding: pure data-parallel over batch — one image per NeuronCore, small
weights replicated; no collectives.

Per-core Bass/Tile program (v4):
  1. offset conv (27ch 3x3) on the PE as 9 shifted bf16 matmuls over padded x
  2. softmax mask + bilinear coefficients + gather indices on DVE/ACT in a
     (pixel-partition, (tile,tap)-free) layout; the wrapped-16 index layout
     dma_gather wants is produced with exact identity-slice matmuls on the
     PE (coords <= 70 stay exact in reduced precision) + strided DVE casts
  3. a 4-corner-duplicated padded image Z (72*72 rows x 4x128ch bf16) is
     staged in DRAM: Z[y,x] = [x(y,x), x(y,x+1), x(y+1,x), x(y+1,x+1)], so a
     single 1KB gather descriptor fetches all 4 bilinear corners of one
     (pixel, tap) sample
  4. per (half, tap): one dma_gather, 7 wide DVE ops (bf16, coefficient
     broadcast along the free dim) combine the corners, 16 PE transposes
     back to (channel, pixel) and 4 accumulating bf16 matmuls with the
     128x128x3x3 weight; bias is added on the PSUM->SBUF copy.

The whole offset->coefficient->index chain is split per image half
(independent tiles) and PE work is emitted in dependency-priority order so
the first gather launches as early as possible; gpsimd descriptor
generation (~16.5us per 2048-index gather) is the pacing resource.
"""

import os
import sys

import numpy as np

sys.path.insert(0, "/opt/trn_rl_repo")

from contextlib import ExitStack

import concourse.bacc as bacc
import concourse.mybir as mybir
import concourse.tile as tile
from concourse._compat import get_trn_type
from concourse.alu_op_type import AluOpType as Alu
from concourse.bass import AP
from concourse.bass_utils import run_bass_kernel_spmd
from concourse import library_config

F32 = mybir.dt.float32
BF16 = mybir.dt.bfloat16
I32 = mybir.dt.int32
I16 = mybir.dt.int16

B = 8
C = 128
H = W = 64
HW = H * W
K2 = 9
PADG = 4
GW = H + 2 * PADG      # 72
GROWS = GW * GW        # 5184
NS = 32
NHALF = 2
SH = NS // NHALF             # 16 s-tiles per half
PIX_PER_HALF = HW // NHALF   # 2048

LAST_EXEC_TIME_NS = None


def _emit(tc):
    nc = tc.nc
    x_d = nc.dram_tensor("x", [C, HW], F32, kind="ExternalInput").ap()
    w_om_d = nc.dram_tensor("w_om", [27, 1152], F32, kind="ExternalInput").ap()
    b_om_d = nc.dram_tensor("b_om", [27, 1], F32, kind="ExternalInput").ap()
    weight_d = nc.dram_tensor("weight", [C, 1152], F32, kind="ExternalInput").ap()
    bias_d = nc.dram_tensor("bias", [C, 1], F32, kind="ExternalInput").ap()
    out_d = nc.dram_tensor("out", [C, HW], F32, kind="ExternalOutput").ap()
    z_d = nc.dram_tensor("z_pad", [GROWS, 512], BF16, kind="ExternalOutput").ap()
    consts_d = nc.dram_tensor("consts", [128, 707], F32, kind="ExternalInput").ap()

    nc.gpsimd.load_library(library_config.mlp)

    ctx = ExitStack()
    with ctx:
        cpool = ctx.enter_context(tc.tile_pool(name="const", bufs=1))
        spool = ctx.enter_context(tc.tile_pool(name="setup", bufs=1))
        dpool = ctx.enter_context(tc.tile_pool(name="data", bufs=1))
        gpool = ctx.enter_context(tc.tile_pool(name="gath", bufs=3))
        vpool = ctx.enter_context(tc.tile_pool(name="val", bufs=2))
        ppool = ctx.enter_context(tc.tile_pool(name="psum", bufs=1, space="PSUM"))
        tpool = ctx.enter_context(tc.tile_pool(name="trps", bufs=2, space="PSUM"))
        opool = ctx.enter_context(tc.tile_pool(name="omps", bufs=2, space="PSUM"))

        # ---------- constants ----------
        cons = cpool.tile([128, 707], F32)
        nc.sync.dma_start(cons[:], consts_d[:, :])
        ident = cons[:, 0:128]
        hob = cons[:, 129:130]
        wo_r = cons[:, 130:131]

        ident_bf = spool.tile([128, 128], BF16)
        nc.vector.tensor_copy(ident_bf[:], ident)

        # ---------- load x; bf16 cast; x_pad ----------
        x_sb = dpool.tile([128, HW], F32, tag="big16k")
        for q in range(4):
            nc.sync.dma_start(x_sb[:, q * 1024:(q + 1) * 1024],
                              x_d[:, q * 1024:(q + 1) * 1024])
        x_bf = spool.tile([128, HW], BF16)
        for q in range(4):
            nc.vector.tensor_copy(x_bf[:, q * 1024:(q + 1) * 1024],
                                  x_sb[:, q * 1024:(q + 1) * 1024])

        XP = 66
        x_pad = spool.tile([128, XP * XP], BF16)
        nc.vector.memset(x_pad[:], 0.0)

        # Z border zero-fill (no deps beyond the memset; fire early)
        zero_bf = spool.tile([128, 1152], BF16)
        nc.vector.memset(zero_bf[:], 0.0)
        nc.sync.dma_start(
            AP(z_d.tensor, 0, [[1152, 128], [1, 1152]]), zero_bf[:])
        nc.sync.dma_start(
            AP(z_d.tensor, (GW - PADG) * GW * 512, [[1152, 128], [1, 1152]]),
            zero_bf[:])
        nc.sync.dma_start(
            AP(z_d.tensor, PADG * GW * 512, [[GW * 512, 64], [1, 2048]]),
            zero_bf[:, 0:1024])
        nc.sync.dma_start(
            AP(z_d.tensor, (PADG * GW + GW - PADG) * 512,
               [[GW * 512, 64], [1, 2048]]),
            zero_bf[:, 0:1024])

        stg_all = spool.tile([128, NS, 128], BF16)

        # ---------- Z interior (before the om chain: the PE transposes can
        # start as soon as the x quarters land, and the corner DMAs use the
        # otherwise-idle early sync window) ----------
        for s in range(NS):
            trp = tpool.tile([128, 128], BF16, tag="trb", name="trp")
            nc.tensor.transpose(trp[:], x_bf[:, s * 128:(s + 1) * 128],
                                ident_bf)
            nc.scalar.copy(stg_all[:, s, :], trp[:])
        # ---------- weights ----------
        w_om_sb = spool.tile([27, 1152], F32)
        nc.sync.dma_start(w_om_sb[:], w_om_d[:, :])
        womb = spool.tile([27, 1152], BF16)
        nc.vector.tensor_copy(womb[:], w_om_sb[:])
        b_om_sb = spool.tile([27, 1], F32)
        nc.sync.dma_start(b_om_sb[:], b_om_d[:, :])
        weight_sb = spool.tile([128, 1152], F32)
        nc.sync.dma_start(weight_sb[:], weight_d[:, :])
        wbf = spool.tile([128, 1152], BF16)
        nc.vector.tensor_copy(wbf[:], weight_sb[:])
        bias_sb = spool.tile([128, 1], F32)
        nc.sync.dma_start(bias_sb[:], bias_d[:, :])

        # dependent sync DMAs after the dependency-free weight loads, so
        # the sync queue never head-of-line blocks them
        for q in range(4):
            nc.sync.dma_start(
                x_pad[:].rearrange("p (a b) -> p a b", a=XP)
                [:, 1 + 16 * q:1 + 16 * (q + 1), 1:65],
                x_bf[:, q * 1024:(q + 1) * 1024])
        for cc, (ry, rx) in enumerate(((0, 0), (0, 1), (1, 0), (1, 1))):
            for r in range(2):
                dst = AP(
                    z_d.tensor,
                    ((r + PADG - ry) * GW + PADG - rx) * 512 + cc * 128,
                    [[512, 64], [2 * GW * 512, NS], [1, 128]],
                )
                nc.sync.dma_start(dst, stg_all[64 * r:64 * r + 64, :, :])


        # womT first: it gates the offset conv (the longest setup chain)
        womT = spool.tile([128, K2, 27], BF16)
        for k in range(K2):
            trp = tpool.tile([128, 128], BF16, tag="trb", name="trp")
            nc.tensor.transpose(
                trp[:, :27], womb[:].rearrange("p (c k) -> p c k", k=K2)[:, :, k],
                ident_bf[0:27, 0:27],
            )
            nc.scalar.copy(womT[:, k, :], trp[:, :27])

        xpv = x_pad[:].rearrange("p (a b) -> p a b", a=XP)

        _cnt = [0]

        def f(shape=(128, SH, K2), dt=F32, tag=None):
            _cnt[0] += 1
            nm = f"cf{_cnt[0]}"
            return dpool.tile(list(shape), dt, tag=tag or nm, name=nm)

        def floorit(v):
            vi = f(dt=I32, tag="fl_i")
            nc.vector.tensor_copy(vi[:], v[:])
            v0 = f(tag="fl_f")
            nc.vector.tensor_copy(v0[:], vi[:])
            gt = f(tag="fl_gt")
            nc.vector.tensor_tensor(gt[:], v0[:], v[:], Alu.is_gt)
            v0f = f()
            nc.vector.tensor_tensor(v0f[:], v0[:], gt[:], Alu.subtract)
            return v0f

        idxw = dpool.tile([128, K2 * 256], I16)
        idxw_t = idxw[:].tensor
        idxw_off = idxw[:].offset

        cbs = []  # per half: (cb00, cb01, cb10, cb11)
        wT = spool.tile([128, K2, 128], BF16)

        def emit_z_and_wt():
            # main-conv weight transposes (Z itself is built early, above)
            for k in range(K2):
                trp = tpool.tile([128, 128], BF16, tag="trb", name="trp")
                nc.tensor.transpose(
                    trp[:], wbf[:].rearrange("p (c k) -> p c k", k=K2)[:, :, k],
                    ident_bf,
                )
                nc.scalar.copy(wT[:, k, :], trp[:])

        for hh in range(NHALF):
            # ---- offset conv for rows [32*hh, 32*hh+32): om in bf16 ----
            om_bf = spool.tile([27, HW // 2], BF16, name=f"om{hh}")
            for chl in range(4):
                ch = 4 * hh + chl
                omp = opool.tile([27, 512], F32, tag="om", name="omp")
                for k in range(K2):
                    dy_, dx_ = k // 3, k % 3
                    r0 = ch * 8 + dy_
                    nc.tensor.matmul(
                        omp[:], womT[:, k, :27], xpv[:, r0:r0 + 8, dx_:dx_ + 64],
                        start=(k == 0), stop=(k == K2 - 1),
                    )
                nc.scalar.activation(
                    om_bf[:, chl * 512:(chl + 1) * 512], omp[:],
                    mybir.ActivationFunctionType.Identity, bias=b_om_sb[:],
                    scale=1.0,
                )

            # ---- omT (128 pix, 27) per s-tile ----
            omT = spool.tile([128, SH, 27], BF16, name=f"omT{hh}")
            for s in range(SH):
                trp = tpool.tile([128, 128], BF16, tag="trb", name="trp")
                nc.tensor.transpose(
                    trp[:, :27], om_bf[:, s * 128:(s + 1) * 128],
                    ident_bf[0:27, 0:27],
                )
                nc.scalar.copy(omT[:, s, :], trp[:, :27])

            # ---- sampling positions (padded coords; tables carry +PADG) ----
            omT_t = omT[:].tensor
            omT_off = omT[:].offset
            dyT = AP(omT_t, omT_off + 0, [[SH * 27, 128], [27, SH], [2, K2]])
            dxT = AP(omT_t, omT_off + 1, [[SH * 27, 128], [27, SH], [2, K2]])
            mlg = omT[:, :, 18:27]
            ykv = cons[:, 131 + 144 * hh:131 + 144 * hh + 144].rearrange(
                "p (s a) -> p s a", a=K2)
            xkv = cons[:, 419 + 144 * hh:419 + 144 * hh + 144].rearrange(
                "p (s a) -> p s a", a=K2)

            py = f()
            nc.vector.scalar_tensor_tensor(py[:], dyT, hob, ykv, Alu.add, Alu.add)
            px = f()
            nc.vector.scalar_tensor_tensor(px[:], dxT, wo_r, xkv, Alu.add, Alu.add)
            y0f = floorit(py)
            x0f = floorit(px)

            # clipped integer corner coords in padded Z space: [0, 70]
            yx = dpool.tile([128, 2, SH, K2], F32, name=f"yx{hh}")
            nc.vector.tensor_scalar(yx[:, 0, :, :], y0f[:], 0.0, float(GW - 2),
                                    Alu.max, Alu.min)
            nc.vector.tensor_scalar(yx[:, 1, :, :], x0f[:], 0.0, float(GW - 2),
                                    Alu.max, Alu.min)

            # ---- idx fold to the gather's wrapped-16 layout ----
            # idxw[16g+m, k*256 + 128*hh + 8s + u] = zrow(pixel of
            #   (s-tile 16*hh+s, lane 16u+m), tap k), for all groups g
            stage = dpool.tile([16, 288], F32, name=f"stg{hh}")
            stage2 = dpool.tile([16, 144], F32, name=f"stg2{hh}")
            for u in range(8):
                pyp = opool.tile([27, 512], F32, tag="om", name="omp")
                nc.tensor.matmul(pyp[0:16, 0:144], ident[:, 16 * u:16 * u + 16],
                                 yx[:, 0, :, :], start=True, stop=True)
                pxp = opool.tile([27, 512], F32, tag="om", name="omp")
                nc.tensor.matmul(pxp[0:16, 0:144], ident[:, 16 * u:16 * u + 16],
                                 yx[:, 1, :, :], start=True, stop=True)
                nc.scalar.copy(stage[:, 0:144], pyp[0:16, 0:144])
                nc.scalar.copy(stage[:, 144:288], pxp[0:16, 0:144])
                # zrow = y*GW + x
                nc.vector.scalar_tensor_tensor(
                    stage2[:], stage[:, 0:144], float(GW), stage[:, 144:288],
                    Alu.mult, Alu.add)
                src = AP(stage2[:].tensor, stage2[:].offset,
                         [[144, 16], [9, SH], [1, K2]])
                dst = AP(idxw_t, idxw_off + 1152 * hh + u,
                         [[K2 * 256, 16], [8, SH], [128, K2]])
                nc.vector.tensor_copy(dst, src)
            for lo, hi in ((16, 32), (32, 64), (64, 128)):
                nc.sync.dma_start(
                    idxw[lo:hi, 1152 * hh:1152 * hh + 1152],
                    idxw[0:hi - lo, 1152 * hh:1152 * hh + 1152])

            # ---- mask + bilinear coefficients (deferred: off the gather
            #      critical path, overlapped with the first gathers) ----
            def make_coeffs(hh=hh, py=py, px=px, y0f=y0f, x0f=x0f, mlg=mlg):
                e = f()
                nc.scalar.activation(e[:], mlg,
                                     mybir.ActivationFunctionType.Exp)
                ssum = f((128, SH, 1))
                nc.vector.tensor_reduce(ssum[:], e[:], mybir.AxisListType.X,
                                        Alu.add)
                rs = f((128, SH, 1))
                nc.vector.reciprocal(rs[:], ssum[:])
                mask = f()
                nc.vector.tensor_tensor(mask[:], e[:],
                                        rs[:].to_broadcast([128, SH, K2]),
                                        Alu.mult)
                wy1 = f()
                nc.vector.tensor_tensor(wy1[:], py[:], y0f[:], Alu.subtract)
                wy0 = f()
                nc.vector.tensor_scalar(wy0[:], wy1[:], -1.0, 1.0, Alu.mult,
                                        Alu.add)
                wx1 = f()
                nc.vector.tensor_tensor(wx1[:], px[:], x0f[:], Alu.subtract)
                wx0 = f()
                nc.vector.tensor_scalar(wx0[:], wx1[:], -1.0, 1.0, Alu.mult,
                                        Alu.add)
                mwy0 = f()
                nc.vector.tensor_tensor(mwy0[:], mask[:], wy0[:], Alu.mult)
                mwy1 = f()
                nc.vector.tensor_tensor(mwy1[:], mask[:], wy1[:], Alu.mult)
                # ccat4[p, s, k, corner] bf16 — corner-interleaved so the
                # main loop can apply all 4 corners in one 4D DVE op
                ccat4 = dpool.tile([128, SH, K2, 4], BF16, name=f"cc4_{hh}")
                cc_t = ccat4[:].tensor
                cc_off = ccat4[:].offset
                for ci, (a_, b_) in enumerate(((mwy0, wx0), (mwy0, wx1),
                                               (mwy1, wx0), (mwy1, wx1))):
                    cf = f()
                    nc.vector.tensor_tensor(cf[:], a_[:], b_[:], Alu.mult)
                    dst = AP(cc_t, cc_off + ci,
                             [[SH * K2 * 4, 128], [K2 * 4, SH], [4, K2]])
                    nc.vector.tensor_copy(dst, cf[:])
                return ccat4

            cbs.append(make_coeffs())

            if hh == 0:
                emit_z_and_wt()

        # ---------- Z (4-corner duplicated padded image) in DRAM ----------
        # ---------- main loop ----------
        out_sb = dpool.tile([128, HW], F32, tag="big16k")
        z_src = AP(z_d.tensor, 0, [[512, GROWS], [1, 512]])
        shp = [128, SH, 128]
        for h in range(NHALF):
            cb4 = cbs[h]
            outp = ppool.tile([128, PIX_PER_HALF], F32, tag="out", name="outp")
            for k in range(K2):
                gZ = gpool.tile([128, SH, 512], BF16, tag="gZ", name="gZ")
                nc.gpsimd.dma_gather(
                    gZ[:], z_src,
                    idxw[:, 1152 * h + 128 * k: 1152 * h + 128 * k + 128],
                    PIX_PER_HALF, PIX_PER_HALF, 512, elem_step=512,
                    single_packet=False,
                )

                def cb(ci):
                    s_ = cb4[:, :, k, ci:ci + 1]
                    return s_.to_broadcast(shp)

                a = vpool.tile(shp, BF16, tag="pa", name="pa")
                nc.vector.tensor_tensor(a[:], gZ[:, :, 0:128], cb(0), Alu.mult)
                b = vpool.tile(shp, BF16, tag="pb", name="pb")
                nc.vector.tensor_tensor(b[:], gZ[:, :, 128:256], cb(1), Alu.mult)
                v = vpool.tile(shp, BF16, tag="pv", name="pv")
                nc.vector.tensor_tensor(v[:], a[:], b[:], Alu.add)
                a2 = vpool.tile(shp, BF16, tag="pa", name="pa")
                nc.vector.tensor_tensor(a2[:], gZ[:, :, 256:384], cb(2), Alu.mult)
                b2 = vpool.tile(shp, BF16, tag="pb", name="pb")
                nc.vector.tensor_tensor(b2[:], gZ[:, :, 384:512], cb(3), Alu.mult)
                v2 = vpool.tile(shp, BF16, tag="pv", name="pv")
                nc.vector.tensor_tensor(v2[:], a2[:], b2[:], Alu.add)
                nc.vector.tensor_tensor(v[:], v[:], v2[:], Alu.add)

                vT = vpool.tile([128, 4 * 512], BF16, tag="vT", name="vT")
                for t in range(SH):
                    trp = tpool.tile([128, 128], BF16, tag="trb", name="trp")
                    nc.tensor.transpose(trp[:], v[:, t, :], ident_bf)
                    nc.scalar.copy(vT[:, t * 128:(t + 1) * 128], trp[:])
                for bk in range(4):
                    nc.tensor.matmul(
                        outp[:, bk * 512:(bk + 1) * 512], wT[:, k, :],
                        vT[:, bk * 512:(bk + 1) * 512],
                        start=(k == 0), stop=(k == K2 - 1),
                    )
            for bk in range(4):
                nc.scalar.activation(
                    out_sb[:, h * PIX_PER_HALF + bk * 512:
                           h * PIX_PER_HALF + (bk + 1) * 512],
                    outp[:, bk * 512:(bk + 1) * 512],
                    mybir.ActivationFunctionType.Identity, bias=bias_sb[:],
                    scale=1.0,
                )
        nc.sync.dma_start(out_d[:, :], out_sb[:])


def _make_consts():
    c = np.zeros((128, 707), np.float32)
    c[:, 0:128] = np.eye(128, dtype=np.float32)
    p = np.arange(128)
    c[:, 128] = p
    c[:, 129] = (p >= 64)
    c[:, 130] = p % 64
    s = np.arange(32)[:, None, None]
    kyv = np.arange(3)[None, :, None]
    kxv = np.arange(3)[None, None, :]
    c[:, 131:419] = np.broadcast_to(
        (2 * s + kyv - 1 + PADG + 0 * kxv).reshape(-1), (128, 288))
    c[:, 419:707] = np.broadcast_to(
        (0 * s + 0 * kyv + kxv - 1 + PADG).reshape(-1), (128, 288))
    return c


_COMPILED = None


def _get_compiled():
    global _COMPILED
    if _COMPILED is None:
        nc = bacc.Bacc(get_trn_type() or "TRN2", target_bir_lowering=False,
                       debug=False, num_devices=B)
        with tile.TileContext(nc) as tc:
            _emit(tc)
        nc.compile()
        _COMPILED = nc
    return _COMPILED


def kernel(x, w_om, b_om, weight, bias):
    global LAST_EXEC_TIME_NS
    x = np.ascontiguousarray(np.asarray(x, dtype=np.float32))
    w_om_f = np.ascontiguousarray(np.asarray(w_om, np.float32).reshape(27, 1152))
    b_om_f = np.ascontiguousarray(np.asarray(b_om, np.float32).reshape(27, 1))
    weight_f = np.ascontiguousarray(np.asarray(weight, np.float32).reshape(128, 1152))
    bias_f = np.ascontiguousarray(np.asarray(bias, np.float32).reshape(128, 1))

    nc = _get_compiled()
    consts = _make_consts()
    in_maps = [
        {
            "x": np.ascontiguousarray(x[b].reshape(C, HW)),
            "w_om": w_om_f,
            "b_om": b_om_f,
            "weight": weight_f,
            "bias": bias_f,
            "consts": consts,
        }
        for b in range(B)
    ]
    trace = bool(os.environ.get("DCN_TRACE"))
    res = run_bass_kernel_spmd(nc, in_maps, core_ids=list(range(B)), trace=trace)
    LAST_EXEC_TIME_NS = res.exec_time_ns
    out = np.stack([res.results[b]["out"].reshape(C, H, W) for b in range(B)])
    return out.astype(np.float32)


# revision 24
# speedup vs baseline: 1.1038x; 1.0042x over previous
"""DCNv2 (modulated deformable convolution) on 8 Trainium2 NeuronCores.

kernel(**inputs) takes the full unsharded inputs
    x      (8, 128, 64, 64) f32
    w_om   (27, 128, 3, 3)  f32
    b_om   (27,)            f32
    weight (128, 128, 3, 3) f32
    bias   (128,)           f32
and returns the full output (8, 128, 64, 64) f32.

Sharding: pure data-parallel over batch — one image per NeuronCore, small
weights replicated; no collectives.

Per-core Bass/Tile program (v4):
  1. offset conv (27ch 3x3) on the PE as 9 shifted bf16 matmuls over padded x
  2. softmax mask + bilinear coefficients + gather indices on DVE/ACT in a
     (pixel-partition, (tile,tap)-free) layout; the wrapped-16 index layout
     dma_gather wants is produced with exact identity-slice matmuls on the
     PE (coords <= 70 stay exact in reduced precision) + strided DVE casts
  3. a 4-corner-duplicated padded image Z (72*72 rows x 4x128ch bf16) is
     staged in DRAM: Z[y,x] = [x(y,x), x(y,x+1), x(y+1,x), x(y+1,x+1)], so a
     single 1KB gather descriptor fetches all 4 bilinear corners of one
     (pixel, tap) sample
  4. per (half, tap): one dma_gather, 7 wide DVE ops (bf16, coefficient
     broadcast along the free dim) combine the corners, 16 PE transposes
     back to (channel, pixel) and 4 accumulating bf16 matmuls with the
     128x128x3x3 weight; bias is added on the PSUM->SBUF copy.

The whole offset->coefficient->index chain is split per image half
(independent tiles) and PE work is emitted in dependency-priority order so
the first gather launches as early as possible; gpsimd descriptor
generation (~16.5us per 2048-index gather) is the pacing resource.
"""

import os
import sys

import numpy as np

sys.path.insert(0, "/opt/trn_rl_repo")

from contextlib import ExitStack

import concourse.bacc as bacc
import concourse.mybir as mybir
import concourse.tile as tile
from concourse._compat import get_trn_type
from concourse.alu_op_type import AluOpType as Alu
from concourse.bass import AP
from concourse.bass_utils import run_bass_kernel_spmd
from concourse import library_config

F32 = mybir.dt.float32
BF16 = mybir.dt.bfloat16
I32 = mybir.dt.int32
I16 = mybir.dt.int16

B = 8
C = 128
H = W = 64
HW = H * W
K2 = 9
PADG = 4
GW = H + 2 * PADG      # 72
GROWS = GW * GW        # 5184
NS = 32
NHALF = 2
SH = NS // NHALF             # 16 s-tiles per half
PIX_PER_HALF = HW // NHALF   # 2048

LAST_EXEC_TIME_NS = None


def _emit(tc):
    nc = tc.nc
    x_d = nc.dram_tensor("x", [C, HW], F32, kind="ExternalInput").ap()
    w_om_d = nc.dram_tensor("w_om", [27, 1152], F32, kind="ExternalInput").ap()
    b_om_d = nc.dram_tensor("b_om", [27, 1], F32, kind="ExternalInput").ap()
    weight_d = nc.dram_tensor("weight", [C, 1152], F32, kind="ExternalInput").ap()
    bias_d = nc.dram_tensor("bias", [C, 1], F32, kind="ExternalInput").ap()
    out_d = nc.dram_tensor("out", [C, HW], F32, kind="ExternalOutput").ap()
    z_d = nc.dram_tensor("z_pad", [GROWS, 512], BF16, kind="ExternalOutput").ap()
    consts_d = nc.dram_tensor("consts", [128, 707], F32, kind="ExternalInput").ap()

    nc.gpsimd.load_library(library_config.mlp)

    ctx = ExitStack()
    with ctx:
        cpool = ctx.enter_context(tc.tile_pool(name="const", bufs=1))
        spool = ctx.enter_context(tc.tile_pool(name="setup", bufs=1))
        dpool = ctx.enter_context(tc.tile_pool(name="data", bufs=1))
        gpool = ctx.enter_context(tc.tile_pool(name="gath", bufs=3))
        vpool = ctx.enter_context(tc.tile_pool(name="val", bufs=2))
        ppool = ctx.enter_context(tc.tile_pool(name="psum", bufs=1, space="PSUM"))
        tpool = ctx.enter_context(tc.tile_pool(name="trps", bufs=2, space="PSUM"))
        opool = ctx.enter_context(tc.tile_pool(name="omps", bufs=2, space="PSUM"))

        # ---------- constants ----------
        cons = cpool.tile([128, 707], F32)
        nc.sync.dma_start(cons[:], consts_d[:, :])
        ident = cons[:, 0:128]
        hob = cons[:, 129:130]
        wo_r = cons[:, 130:131]

        ident_bf = spool.tile([128, 128], BF16)
        nc.vector.tensor_copy(ident_bf[:], ident)

        # ---------- load x; bf16 cast; x_pad ----------
        x_sb = dpool.tile([128, HW], F32, tag="big16k")
        for q in range(4):
            nc.sync.dma_start(x_sb[:, q * 1024:(q + 1) * 1024],
                              x_d[:, q * 1024:(q + 1) * 1024])
        x_bf = spool.tile([128, HW], BF16)
        for q in range(4):
            nc.vector.tensor_copy(x_bf[:, q * 1024:(q + 1) * 1024],
                                  x_sb[:, q * 1024:(q + 1) * 1024])

        XP = 66
        x_pad = spool.tile([128, XP * XP], BF16)
        nc.vector.memset(x_pad[:], 0.0)

        # Z border zero-fill (no deps beyond the memset; fire early)
        zero_bf = spool.tile([128, 1152], BF16)
        nc.vector.memset(zero_bf[:], 0.0)
        nc.sync.dma_start(
            AP(z_d.tensor, 0, [[1152, 128], [1, 1152]]), zero_bf[:])
        nc.sync.dma_start(
            AP(z_d.tensor, (GW - PADG) * GW * 512, [[1152, 128], [1, 1152]]),
            zero_bf[:])
        nc.sync.dma_start(
            AP(z_d.tensor, PADG * GW * 512, [[GW * 512, 64], [1, 2048]]),
            zero_bf[:, 0:1024])
        nc.sync.dma_start(
            AP(z_d.tensor, (PADG * GW + GW - PADG) * 512,
               [[GW * 512, 64], [1, 2048]]),
            zero_bf[:, 0:1024])

        stg_all = spool.tile([128, NS, 128], BF16)

        # ---------- Z interior (before the om chain: the PE transposes can
        # start as soon as the x quarters land, and the corner DMAs use the
        # otherwise-idle early sync window) ----------
        for s in range(NS):
            trp = tpool.tile([128, 128], BF16, tag="trb", name="trp")
            nc.tensor.transpose(trp[:], x_bf[:, s * 128:(s + 1) * 128],
                                ident_bf)
            nc.scalar.copy(stg_all[:, s, :], trp[:])
        # ---------- weights ----------
        w_om_sb = spool.tile([27, 1152], F32)
        nc.sync.dma_start(w_om_sb[:], w_om_d[:, :])
        womb = spool.tile([27, 1152], BF16)
        nc.vector.tensor_copy(womb[:], w_om_sb[:])
        b_om_sb = spool.tile([27, 1], F32)
        nc.sync.dma_start(b_om_sb[:], b_om_d[:, :])
        weight_sb = spool.tile([128, 1152], F32)
        nc.sync.dma_start(weight_sb[:], weight_d[:, :])
        wbf = spool.tile([128, 1152], BF16)
        nc.vector.tensor_copy(wbf[:], weight_sb[:])
        bias_sb = spool.tile([128, 1], F32)
        nc.sync.dma_start(bias_sb[:], bias_d[:, :])

        # dependent sync DMAs after the dependency-free weight loads, so
        # the sync queue never head-of-line blocks them
        for q in range(4):
            nc.sync.dma_start(
                x_pad[:].rearrange("p (a b) -> p a b", a=XP)
                [:, 1 + 16 * q:1 + 16 * (q + 1), 1:65],
                x_bf[:, q * 1024:(q + 1) * 1024])
        for cc, (ry, rx) in enumerate(((0, 0), (0, 1), (1, 0), (1, 1))):
            for r in range(2):
                dst = AP(
                    z_d.tensor,
                    ((r + PADG - ry) * GW + PADG - rx) * 512 + cc * 128,
                    [[512, 64], [2 * GW * 512, NS], [1, 128]],
                )
                nc.sync.dma_start(dst, stg_all[64 * r:64 * r + 64, :, :])


        # womT first: it gates the offset conv (the longest setup chain)
        womT = spool.tile([128, K2, 27], BF16)
        for k in range(K2):
            trp = tpool.tile([128, 128], BF16, tag="trb", name="trp")
            nc.tensor.transpose(
                trp[:, :27], womb[:].rearrange("p (c k) -> p c k", k=K2)[:, :, k],
                ident_bf[0:27, 0:27],
            )
            nc.scalar.copy(womT[:, k, :], trp[:, :27])

        xpv = x_pad[:].rearrange("p (a b) -> p a b", a=XP)

        _cnt = [0]

        def f(shape=(128, SH, K2), dt=F32, tag=None):
            _cnt[0] += 1
            nm = f"cf{_cnt[0]}"
            return dpool.tile(list(shape), dt, tag=tag or nm, name=nm)

        def floorit(v):
            vi = f(dt=I32, tag="fl_i")
            nc.vector.tensor_copy(vi[:], v[:])
            v0 = f(tag="fl_f")
            nc.vector.tensor_copy(v0[:], vi[:])
            gt = f(tag="fl_gt")
            nc.vector.tensor_tensor(gt[:], v0[:], v[:], Alu.is_gt)
            v0f = f()
            nc.vector.tensor_tensor(v0f[:], v0[:], gt[:], Alu.subtract)
            return v0f

        idxw = dpool.tile([128, K2 * 256], I16)
        idxw_t = idxw[:].tensor
        idxw_off = idxw[:].offset

        cbs = []  # per half: (cb00, cb01, cb10, cb11)
        wT = spool.tile([128, K2, 128], BF16)

        def emit_z_and_wt():
            # main-conv weight transposes (Z itself is built early, above)
            for k in range(K2):
                trp = tpool.tile([128, 128], BF16, tag="trb", name="trp")
                nc.tensor.transpose(
                    trp[:], wbf[:].rearrange("p (c k) -> p c k", k=K2)[:, :, k],
                    ident_bf,
                )
                nc.scalar.copy(wT[:, k, :], trp[:])

        for hh in range(NHALF):
            # ---- offset conv for rows [32*hh, 32*hh+32): om in bf16 ----
            om_bf = spool.tile([27, HW // 2], BF16, name=f"om{hh}")
            for chl in range(4):
                ch = 4 * hh + chl
                omp = opool.tile([27, 512], F32, tag="om", name="omp")
                for k in range(K2):
                    dy_, dx_ = k // 3, k % 3
                    r0 = ch * 8 + dy_
                    nc.tensor.matmul(
                        omp[:], womT[:, k, :27], xpv[:, r0:r0 + 8, dx_:dx_ + 64],
                        start=(k == 0), stop=(k == K2 - 1),
                    )
                nc.scalar.activation(
                    om_bf[:, chl * 512:(chl + 1) * 512], omp[:],
                    mybir.ActivationFunctionType.Identity, bias=b_om_sb[:],
                    scale=1.0,
                )

            # ---- omT (128 pix, 27) per s-tile ----
            omT = spool.tile([128, SH, 27], BF16, name=f"omT{hh}")
            for s in range(SH):
                trp = tpool.tile([128, 128], BF16, tag="trb", name="trp")
                nc.tensor.transpose(
                    trp[:, :27], om_bf[:, s * 128:(s + 1) * 128],
                    ident_bf[0:27, 0:27],
                )
                nc.scalar.copy(omT[:, s, :], trp[:, :27])

            # ---- sampling positions (padded coords; tables carry +PADG) ----
            omT_t = omT[:].tensor
            omT_off = omT[:].offset
            dyT = AP(omT_t, omT_off + 0, [[SH * 27, 128], [27, SH], [2, K2]])
            dxT = AP(omT_t, omT_off + 1, [[SH * 27, 128], [27, SH], [2, K2]])
            mlg = omT[:, :, 18:27]
            ykv = cons[:, 131 + 144 * hh:131 + 144 * hh + 144].rearrange(
                "p (s a) -> p s a", a=K2)
            xkv = cons[:, 419 + 144 * hh:419 + 144 * hh + 144].rearrange(
                "p (s a) -> p s a", a=K2)

            py = f()
            nc.vector.scalar_tensor_tensor(py[:], dyT, hob, ykv, Alu.add, Alu.add)
            px = f()
            nc.vector.scalar_tensor_tensor(px[:], dxT, wo_r, xkv, Alu.add, Alu.add)
            y0f = floorit(py)
            x0f = floorit(px)

            # clipped integer corner coords in padded Z space: [0, 70]
            yx = dpool.tile([128, 2, SH, K2], F32, name=f"yx{hh}")
            nc.vector.tensor_scalar(yx[:, 0, :, :], y0f[:], 0.0, float(GW - 2),
                                    Alu.max, Alu.min)
            nc.vector.tensor_scalar(yx[:, 1, :, :], x0f[:], 0.0, float(GW - 2),
                                    Alu.max, Alu.min)

            # ---- idx fold to the gather's wrapped-16 layout ----
            # idxw[16g+m, k*256 + 128*hh + 8s + u] = zrow(pixel of
            #   (s-tile 16*hh+s, lane 16u+m), tap k), for all groups g
            stage = dpool.tile([16, 288], F32, name=f"stg{hh}")
            stage2 = dpool.tile([16, 144], F32, name=f"stg2{hh}")
            for u in range(8):
                pyp = opool.tile([27, 512], F32, tag="om", name="omp")
                nc.tensor.matmul(pyp[0:16, 0:288], ident[:, 16 * u:16 * u + 16],
                                 yx[:, :, :, :], start=True, stop=True)
                nc.scalar.copy(stage[:, 0:288], pyp[0:16, 0:288])
                # zrow = y*GW + x
                nc.vector.scalar_tensor_tensor(
                    stage2[:], stage[:, 0:144], float(GW), stage[:, 144:288],
                    Alu.mult, Alu.add)
                src = AP(stage2[:].tensor, stage2[:].offset,
                         [[144, 16], [9, SH], [1, K2]])
                dst = AP(idxw_t, idxw_off + 1152 * hh + u,
                         [[K2 * 256, 16], [8, SH], [128, K2]])
                nc.vector.tensor_copy(dst, src)
            for lo, hi in ((16, 32), (32, 64), (64, 128)):
                nc.sync.dma_start(
                    idxw[lo:hi, 1152 * hh:1152 * hh + 1152],
                    idxw[0:hi - lo, 1152 * hh:1152 * hh + 1152])

            # ---- mask + bilinear coefficients (deferred: off the gather
            #      critical path, overlapped with the first gathers) ----
            def make_coeffs(hh=hh, py=py, px=px, y0f=y0f, x0f=x0f, mlg=mlg):
                e = f()
                nc.scalar.activation(e[:], mlg,
                                     mybir.ActivationFunctionType.Exp)
                ssum = f((128, SH, 1))
                nc.vector.tensor_reduce(ssum[:], e[:], mybir.AxisListType.X,
                                        Alu.add)
                rs = f((128, SH, 1))
                nc.vector.reciprocal(rs[:], ssum[:])
                mask = f()
                nc.vector.tensor_tensor(mask[:], e[:],
                                        rs[:].to_broadcast([128, SH, K2]),
                                        Alu.mult)
                wy1 = f()
                nc.vector.tensor_tensor(wy1[:], py[:], y0f[:], Alu.subtract)
                wy0 = f()
                nc.vector.tensor_scalar(wy0[:], wy1[:], -1.0, 1.0, Alu.mult,
                                        Alu.add)
                wx1 = f()
                nc.vector.tensor_tensor(wx1[:], px[:], x0f[:], Alu.subtract)
                wx0 = f()
                nc.vector.tensor_scalar(wx0[:], wx1[:], -1.0, 1.0, Alu.mult,
                                        Alu.add)
                mwy0 = f()
                nc.vector.tensor_tensor(mwy0[:], mask[:], wy0[:], Alu.mult)
                mwy1 = f()
                nc.vector.tensor_tensor(mwy1[:], mask[:], wy1[:], Alu.mult)
                # ccat4[p, s, k, corner] bf16 — corner-interleaved so the
                # main loop can apply all 4 corners in one 4D DVE op
                ccat4 = dpool.tile([128, SH, K2, 4], BF16, name=f"cc4_{hh}")
                cc_t = ccat4[:].tensor
                cc_off = ccat4[:].offset
                for ci, (a_, b_) in enumerate(((mwy0, wx0), (mwy0, wx1),
                                               (mwy1, wx0), (mwy1, wx1))):
                    cf = f()
                    nc.vector.tensor_tensor(cf[:], a_[:], b_[:], Alu.mult)
                    dst = AP(cc_t, cc_off + ci,
                             [[SH * K2 * 4, 128], [K2 * 4, SH], [4, K2]])
                    nc.vector.tensor_copy(dst, cf[:])
                return ccat4

            cbs.append(make_coeffs())

            if hh == 0:
                emit_z_and_wt()

        # ---------- Z (4-corner duplicated padded image) in DRAM ----------
        # ---------- main loop ----------
        out_sb = dpool.tile([128, HW], F32, tag="big16k")
        z_src = AP(z_d.tensor, 0, [[512, GROWS], [1, 512]])
        shp = [128, SH, 128]
        for h in range(NHALF):
            cb4 = cbs[h]
            outp = ppool.tile([128, PIX_PER_HALF], F32, tag="out", name="outp")
            for k in range(K2):
                gZ = gpool.tile([128, SH, 512], BF16, tag="gZ", name="gZ")
                nc.gpsimd.dma_gather(
                    gZ[:], z_src,
                    idxw[:, 1152 * h + 128 * k: 1152 * h + 128 * k + 128],
                    PIX_PER_HALF, PIX_PER_HALF, 512, elem_step=512,
                    single_packet=False,
                )

                def cb(ci):
                    s_ = cb4[:, :, k, ci:ci + 1]
                    return s_.to_broadcast(shp)

                a = vpool.tile(shp, BF16, tag="pa", name="pa")
                nc.vector.tensor_tensor(a[:], gZ[:, :, 0:128], cb(0), Alu.mult)
                b = vpool.tile(shp, BF16, tag="pb", name="pb")
                nc.vector.tensor_tensor(b[:], gZ[:, :, 128:256], cb(1), Alu.mult)
                v = vpool.tile(shp, BF16, tag="pv", name="pv")
                nc.vector.tensor_tensor(v[:], a[:], b[:], Alu.add)
                a2 = vpool.tile(shp, BF16, tag="pa", name="pa")
                nc.vector.tensor_tensor(a2[:], gZ[:, :, 256:384], cb(2), Alu.mult)
                b2 = vpool.tile(shp, BF16, tag="pb", name="pb")
                nc.vector.tensor_tensor(b2[:], gZ[:, :, 384:512], cb(3), Alu.mult)
                v2 = vpool.tile(shp, BF16, tag="pv", name="pv")
                nc.vector.tensor_tensor(v2[:], a2[:], b2[:], Alu.add)
                nc.vector.tensor_tensor(v[:], v[:], v2[:], Alu.add)

                vT = vpool.tile([128, 4 * 512], BF16, tag="vT", name="vT")
                for t in range(SH):
                    trp = tpool.tile([128, 128], BF16, tag="trb", name="trp")
                    nc.tensor.transpose(trp[:], v[:, t, :], ident_bf)
                    nc.scalar.copy(vT[:, t * 128:(t + 1) * 128], trp[:])
                for bk in range(4):
                    nc.tensor.matmul(
                        outp[:, bk * 512:(bk + 1) * 512], wT[:, k, :],
                        vT[:, bk * 512:(bk + 1) * 512],
                        start=(k == 0), stop=(k == K2 - 1),
                    )
            for bk in range(4):
                nc.scalar.activation(
                    out_sb[:, h * PIX_PER_HALF + bk * 512:
                           h * PIX_PER_HALF + (bk + 1) * 512],
                    outp[:, bk * 512:(bk + 1) * 512],
                    mybir.ActivationFunctionType.Identity, bias=bias_sb[:],
                    scale=1.0,
                )
        for q in range(4):
            nc.sync.dma_start(out_d[:, q * 1024:(q + 1) * 1024],
                              out_sb[:, q * 1024:(q + 1) * 1024])


def _make_consts():
    c = np.zeros((128, 707), np.float32)
    c[:, 0:128] = np.eye(128, dtype=np.float32)
    p = np.arange(128)
    c[:, 128] = p
    c[:, 129] = (p >= 64)
    c[:, 130] = p % 64
    s = np.arange(32)[:, None, None]
    kyv = np.arange(3)[None, :, None]
    kxv = np.arange(3)[None, None, :]
    c[:, 131:419] = np.broadcast_to(
        (2 * s + kyv - 1 + PADG + 0 * kxv).reshape(-1), (128, 288))
    c[:, 419:707] = np.broadcast_to(
        (0 * s + 0 * kyv + kxv - 1 + PADG).reshape(-1), (128, 288))
    return c


_COMPILED = None


def _get_compiled():
    global _COMPILED
    if _COMPILED is None:
        nc = bacc.Bacc(get_trn_type() or "TRN2", target_bir_lowering=False,
                       debug=False, num_devices=B)
        with tile.TileContext(nc) as tc:
            _emit(tc)
        nc.compile()
        _COMPILED = nc
    return _COMPILED


def kernel(x, w_om, b_om, weight, bias):
    global LAST_EXEC_TIME_NS
    x = np.ascontiguousarray(np.asarray(x, dtype=np.float32))
    w_om_f = np.ascontiguousarray(np.asarray(w_om, np.float32).reshape(27, 1152))
    b_om_f = np.ascontiguousarray(np.asarray(b_om, np.float32).reshape(27, 1))
    weight_f = np.ascontiguousarray(np.asarray(weight, np.float32).reshape(128, 1152))
    bias_f = np.ascontiguousarray(np.asarray(bias, np.float32).reshape(128, 1))

    nc = _get_compiled()
    consts = _make_consts()
    in_maps = [
        {
            "x": np.ascontiguousarray(x[b].reshape(C, HW)),
            "w_om": w_om_f,
            "b_om": b_om_f,
            "weight": weight_f,
            "bias": bias_f,
            "consts": consts,
        }
        for b in range(B)
    ]
    trace = bool(os.environ.get("DCN_TRACE"))
    res = run_bass_kernel_spmd(nc, in_maps, core_ids=list(range(B)), trace=trace)
    LAST_EXEC_TIME_NS = res.exec_time_ns
    out = np.stack([res.results[b]["out"].reshape(C, H, W) for b in range(B)])
    return out.astype(np.float32)


# revision 31
# speedup vs baseline: 1.1087x; 1.0044x over previous
"""DCNv2 (modulated deformable convolution) on 8 Trainium2 NeuronCores.

kernel(**inputs) takes the full unsharded inputs
    x      (8, 128, 64, 64) f32
    w_om   (27, 128, 3, 3)  f32
    b_om   (27,)            f32
    weight (128, 128, 3, 3) f32
    bias   (128,)           f32
and returns the full output (8, 128, 64, 64) f32.

Sharding: pure data-parallel over batch — one image per NeuronCore, small
weights replicated; no collectives.

Per-core Bass/Tile program (v4):
  1. offset conv (27ch 3x3) on the PE as 9 shifted bf16 matmuls over padded x
  2. softmax mask + bilinear coefficients + gather indices on DVE/ACT in a
     (pixel-partition, (tile,tap)-free) layout; the wrapped-16 index layout
     dma_gather wants is produced with exact identity-slice matmuls on the
     PE (coords <= 70 stay exact in reduced precision) + strided DVE casts
  3. a 4-corner-duplicated padded image Z (72*72 rows x 4x128ch bf16) is
     staged in DRAM: Z[y,x] = [x(y,x), x(y,x+1), x(y+1,x), x(y+1,x+1)], so a
     single 1KB gather descriptor fetches all 4 bilinear corners of one
     (pixel, tap) sample
  4. per (half, tap): one dma_gather, 7 wide DVE ops (bf16, coefficient
     broadcast along the free dim) combine the corners, 16 PE transposes
     back to (channel, pixel) and 4 accumulating bf16 matmuls with the
     128x128x3x3 weight; bias is added on the PSUM->SBUF copy.

The whole offset->coefficient->index chain is split per image half
(independent tiles) and PE work is emitted in dependency-priority order so
the first gather launches as early as possible; gpsimd descriptor
generation (~16.5us per 2048-index gather) is the pacing resource.
"""

import os
import sys

import numpy as np

sys.path.insert(0, "/opt/trn_rl_repo")

from contextlib import ExitStack

import concourse.bacc as bacc
import concourse.mybir as mybir
import concourse.tile as tile
from concourse._compat import get_trn_type
from concourse.alu_op_type import AluOpType as Alu
from concourse.bass import AP
from concourse.bass_utils import run_bass_kernel_spmd
from concourse import library_config

F32 = mybir.dt.float32
BF16 = mybir.dt.bfloat16
I32 = mybir.dt.int32
I16 = mybir.dt.int16

B = 8
C = 128
H = W = 64
HW = H * W
K2 = 9
PADG = 4
GW = H + 2 * PADG      # 72
GROWS = GW * GW        # 5184
NS = 32
NHALF = 2
SH = NS // NHALF             # 16 s-tiles per half
PIX_PER_HALF = HW // NHALF   # 2048

LAST_EXEC_TIME_NS = None


def _emit(tc):
    nc = tc.nc
    x_d = nc.dram_tensor("x", [C, HW], F32, kind="ExternalInput").ap()
    w_om_d = nc.dram_tensor("w_om", [27, 1152], F32, kind="ExternalInput").ap()
    b_om_d = nc.dram_tensor("b_om", [27, 1], F32, kind="ExternalInput").ap()
    weight_d = nc.dram_tensor("weight", [C, 1152], F32, kind="ExternalInput").ap()
    bias_d = nc.dram_tensor("bias", [C, 1], F32, kind="ExternalInput").ap()
    out_d = nc.dram_tensor("out", [C, HW], F32, kind="ExternalOutput").ap()
    z_d = nc.dram_tensor("z_pad", [GROWS, 512], BF16, kind="ExternalOutput").ap()
    consts_d = nc.dram_tensor("consts", [128, 707], F32, kind="ExternalInput").ap()

    nc.gpsimd.load_library(library_config.mlp)

    ctx = ExitStack()
    with ctx:
        cpool = ctx.enter_context(tc.tile_pool(name="const", bufs=1))
        spool = ctx.enter_context(tc.tile_pool(name="setup", bufs=1))
        dpool = ctx.enter_context(tc.tile_pool(name="data", bufs=1))
        gpool = ctx.enter_context(tc.tile_pool(name="gath", bufs=3))
        vpool = ctx.enter_context(tc.tile_pool(name="val", bufs=2))
        ppool = ctx.enter_context(tc.tile_pool(name="psum", bufs=1, space="PSUM"))
        tpool = ctx.enter_context(tc.tile_pool(name="trps", bufs=2, space="PSUM"))
        opool = ctx.enter_context(tc.tile_pool(name="omps", bufs=2, space="PSUM"))

        # ---------- constants ----------
        cons = cpool.tile([128, 707], F32)
        nc.sync.dma_start(cons[:], consts_d[:, :])
        ident = cons[:, 0:128]
        hob = cons[:, 129:130]
        wo_r = cons[:, 130:131]

        ident_bf = spool.tile([128, 128], BF16)
        nc.vector.tensor_copy(ident_bf[:], ident)

        # ---------- load x; bf16 cast; x_pad ----------
        x_sb = dpool.tile([128, HW], F32, tag="big16k")
        for q in range(4):
            nc.sync.dma_start(x_sb[:, q * 1024:(q + 1) * 1024],
                              x_d[:, q * 1024:(q + 1) * 1024])
        x_bf = spool.tile([128, HW], BF16)
        for q in range(4):
            nc.vector.tensor_copy(x_bf[:, q * 1024:(q + 1) * 1024],
                                  x_sb[:, q * 1024:(q + 1) * 1024])

        XP = 66
        x_pad = spool.tile([128, XP * XP], BF16)
        nc.vector.memset(x_pad[:], 0.0)

        # Z border zero-fill (no deps beyond the memset; fire early)
        zero_bf = spool.tile([128, 1152], BF16)
        nc.vector.memset(zero_bf[:], 0.0)
        nc.sync.dma_start(
            AP(z_d.tensor, 0, [[1152, 128], [1, 1152]]), zero_bf[:])
        nc.sync.dma_start(
            AP(z_d.tensor, (GW - PADG) * GW * 512, [[1152, 128], [1, 1152]]),
            zero_bf[:])
        nc.sync.dma_start(
            AP(z_d.tensor, PADG * GW * 512, [[GW * 512, 64], [1, 2048]]),
            zero_bf[:, 0:1024])
        nc.sync.dma_start(
            AP(z_d.tensor, (PADG * GW + GW - PADG) * 512,
               [[GW * 512, 64], [1, 2048]]),
            zero_bf[:, 0:1024])

        stg_all = spool.tile([128, NS, 128], BF16)

        # ---------- Z interior (before the om chain: the PE transposes can
        # start as soon as the x quarters land, and the corner DMAs use the
        # otherwise-idle early sync window) ----------
        for s in range(NS):
            trp = tpool.tile([128, 128], BF16, tag="trb", name="trp")
            nc.tensor.transpose(trp[:], x_bf[:, s * 128:(s + 1) * 128],
                                ident_bf)
            nc.vector.tensor_copy(stg_all[:, s, :], trp[:])
        # ---------- weights ----------
        w_om_sb = spool.tile([27, 1152], F32)
        nc.sync.dma_start(w_om_sb[:], w_om_d[:, :])
        womb = spool.tile([27, 1152], BF16)
        nc.vector.tensor_copy(womb[:], w_om_sb[:])
        b_om_sb = spool.tile([27, 1], F32)
        nc.sync.dma_start(b_om_sb[:], b_om_d[:, :])
        weight_sb = spool.tile([128, 1152], F32)
        nc.sync.dma_start(weight_sb[:], weight_d[:, :])
        wbf = spool.tile([128, 1152], BF16)
        nc.vector.tensor_copy(wbf[:], weight_sb[:])
        bias_sb = spool.tile([128, 1], F32)
        nc.sync.dma_start(bias_sb[:], bias_d[:, :])

        # dependent sync DMAs after the dependency-free weight loads, so
        # the sync queue never head-of-line blocks them
        for q in range(4):
            nc.sync.dma_start(
                x_pad[:].rearrange("p (a b) -> p a b", a=XP)
                [:, 1 + 16 * q:1 + 16 * (q + 1), 1:65],
                x_bf[:, q * 1024:(q + 1) * 1024])
        for cc, (ry, rx) in enumerate(((0, 0), (0, 1), (1, 0), (1, 1))):
            for r in range(2):
                dst = AP(
                    z_d.tensor,
                    ((r + PADG - ry) * GW + PADG - rx) * 512 + cc * 128,
                    [[512, 64], [2 * GW * 512, NS], [1, 128]],
                )
                nc.sync.dma_start(dst, stg_all[64 * r:64 * r + 64, :, :])


        # womT first: it gates the offset conv (the longest setup chain)
        womT = spool.tile([128, K2, 27], BF16)
        for k in range(K2):
            trp = tpool.tile([128, 128], BF16, tag="trb", name="trp")
            nc.tensor.transpose(
                trp[:, :27], womb[:].rearrange("p (c k) -> p c k", k=K2)[:, :, k],
                ident_bf[0:27, 0:27],
            )
            nc.scalar.copy(womT[:, k, :], trp[:, :27])

        xpv = x_pad[:].rearrange("p (a b) -> p a b", a=XP)

        _cnt = [0]

        def f(shape=(128, SH, K2), dt=F32, tag=None):
            _cnt[0] += 1
            nm = f"cf{_cnt[0]}"
            return dpool.tile(list(shape), dt, tag=tag or nm, name=nm)

        def floorit(v):
            vi = f(dt=I32, tag="fl_i")
            nc.vector.tensor_copy(vi[:], v[:])
            v0 = f(tag="fl_f")
            nc.vector.tensor_copy(v0[:], vi[:])
            gt = f(tag="fl_gt")
            nc.vector.tensor_tensor(gt[:], v0[:], v[:], Alu.is_gt)
            v0f = f()
            nc.vector.tensor_tensor(v0f[:], v0[:], gt[:], Alu.subtract)
            return v0f

        idxw = dpool.tile([128, K2 * 256], I16)
        idxw_t = idxw[:].tensor
        idxw_off = idxw[:].offset

        cbs = []  # per half: (cb00, cb01, cb10, cb11)
        wT = spool.tile([128, K2, 128], BF16)

        def emit_z_and_wt():
            # main-conv weight transposes (Z itself is built early, above)
            for k in range(K2):
                trp = tpool.tile([128, 128], BF16, tag="trb", name="trp")
                nc.tensor.transpose(
                    trp[:], wbf[:].rearrange("p (c k) -> p c k", k=K2)[:, :, k],
                    ident_bf,
                )
                nc.scalar.copy(wT[:, k, :], trp[:])

        for hh in range(NHALF):
            # ---- offset conv for rows [32*hh, 32*hh+32): om in bf16 ----
            om_bf = spool.tile([27, HW // 2], BF16, name=f"om{hh}")
            for chl in range(4):
                ch = 4 * hh + chl
                omp = opool.tile([27, 512], F32, tag="om", name="omp")
                for k in range(K2):
                    dy_, dx_ = k // 3, k % 3
                    r0 = ch * 8 + dy_
                    nc.tensor.matmul(
                        omp[:], womT[:, k, :27], xpv[:, r0:r0 + 8, dx_:dx_ + 64],
                        start=(k == 0), stop=(k == K2 - 1),
                    )
                nc.scalar.activation(
                    om_bf[:, chl * 512:(chl + 1) * 512], omp[:],
                    mybir.ActivationFunctionType.Identity, bias=b_om_sb[:],
                    scale=1.0,
                )

            # ---- omT (128 pix, 27) per s-tile ----
            omT = spool.tile([128, SH, 27], BF16, name=f"omT{hh}")
            for s in range(SH):
                trp = tpool.tile([128, 128], BF16, tag="trb", name="trp")
                nc.tensor.transpose(
                    trp[:, :27], om_bf[:, s * 128:(s + 1) * 128],
                    ident_bf[0:27, 0:27],
                )
                nc.scalar.copy(omT[:, s, :], trp[:, :27])

            # ---- sampling positions (padded coords; tables carry +PADG) ----
            omT_t = omT[:].tensor
            omT_off = omT[:].offset
            dyT = AP(omT_t, omT_off + 0, [[SH * 27, 128], [27, SH], [2, K2]])
            dxT = AP(omT_t, omT_off + 1, [[SH * 27, 128], [27, SH], [2, K2]])
            mlg = omT[:, :, 18:27]
            ykv = cons[:, 131 + 144 * hh:131 + 144 * hh + 144].rearrange(
                "p (s a) -> p s a", a=K2)
            xkv = cons[:, 419 + 144 * hh:419 + 144 * hh + 144].rearrange(
                "p (s a) -> p s a", a=K2)

            py = f()
            nc.vector.scalar_tensor_tensor(py[:], dyT, hob, ykv, Alu.add, Alu.add)
            px = f()
            nc.vector.scalar_tensor_tensor(px[:], dxT, wo_r, xkv, Alu.add, Alu.add)
            y0f = floorit(py)
            x0f = floorit(px)

            # clipped integer corner coords in padded Z space: [0, 70]
            yx = dpool.tile([128, 2, SH, K2], F32, name=f"yx{hh}")
            nc.vector.tensor_scalar(yx[:, 0, :, :], y0f[:], 0.0, float(GW - 2),
                                    Alu.max, Alu.min)
            nc.vector.tensor_scalar(yx[:, 1, :, :], x0f[:], 0.0, float(GW - 2),
                                    Alu.max, Alu.min)

            # ---- idx fold to the gather's wrapped-16 layout ----
            # idxw[16g+m, k*256 + 128*hh + 8s + u] = zrow(pixel of
            #   (s-tile 16*hh+s, lane 16u+m), tap k), for all groups g
            stage = dpool.tile([16, 288], F32, name=f"stg{hh}")
            stage2 = dpool.tile([16, 144], F32, name=f"stg2{hh}")
            for u in range(8):
                pyp = opool.tile([27, 512], F32, tag="om", name="omp")
                nc.tensor.matmul(pyp[0:16, 0:288], ident[:, 16 * u:16 * u + 16],
                                 yx[:, :, :, :], start=True, stop=True)
                nc.scalar.copy(stage[:, 0:288], pyp[0:16, 0:288])
                # zrow = y*GW + x
                nc.vector.scalar_tensor_tensor(
                    stage2[:], stage[:, 0:144], float(GW), stage[:, 144:288],
                    Alu.mult, Alu.add)
                src = AP(stage2[:].tensor, stage2[:].offset,
                         [[144, 16], [9, SH], [1, K2]])
                dst = AP(idxw_t, idxw_off + 1152 * hh + u,
                         [[K2 * 256, 16], [8, SH], [128, K2]])
                nc.vector.tensor_copy(dst, src)
            for lo, hi in ((16, 32), (32, 64), (64, 128)):
                nc.sync.dma_start(
                    idxw[lo:hi, 1152 * hh:1152 * hh + 1152],
                    idxw[0:hi - lo, 1152 * hh:1152 * hh + 1152])

            # ---- mask + bilinear coefficients (deferred: off the gather
            #      critical path, overlapped with the first gathers) ----
            def make_coeffs(hh=hh, py=py, px=px, y0f=y0f, x0f=x0f, mlg=mlg):
                e = f()
                nc.scalar.activation(e[:], mlg,
                                     mybir.ActivationFunctionType.Exp)
                ssum = f((128, SH, 1))
                nc.vector.tensor_reduce(ssum[:], e[:], mybir.AxisListType.X,
                                        Alu.add)
                rs = f((128, SH, 1))
                nc.vector.reciprocal(rs[:], ssum[:])
                mask = f()
                nc.vector.tensor_tensor(mask[:], e[:],
                                        rs[:].to_broadcast([128, SH, K2]),
                                        Alu.mult)
                wy1 = f()
                nc.vector.tensor_tensor(wy1[:], py[:], y0f[:], Alu.subtract)
                wy0 = f()
                nc.vector.tensor_scalar(wy0[:], wy1[:], -1.0, 1.0, Alu.mult,
                                        Alu.add)
                wx1 = f()
                nc.vector.tensor_tensor(wx1[:], px[:], x0f[:], Alu.subtract)
                wx0 = f()
                nc.vector.tensor_scalar(wx0[:], wx1[:], -1.0, 1.0, Alu.mult,
                                        Alu.add)
                mwy0 = f()
                nc.vector.tensor_tensor(mwy0[:], mask[:], wy0[:], Alu.mult)
                mwy1 = f()
                nc.vector.tensor_tensor(mwy1[:], mask[:], wy1[:], Alu.mult)
                # ccat4[p, s, k, corner] bf16 — corner-interleaved so the
                # main loop can apply all 4 corners in one 4D DVE op
                ccat4 = dpool.tile([128, SH, K2, 4], BF16, name=f"cc4_{hh}")
                cc_t = ccat4[:].tensor
                cc_off = ccat4[:].offset
                for ci, (a_, b_) in enumerate(((mwy0, wx0), (mwy0, wx1),
                                               (mwy1, wx0), (mwy1, wx1))):
                    cf = f()
                    nc.vector.tensor_tensor(cf[:], a_[:], b_[:], Alu.mult)
                    dst = AP(cc_t, cc_off + ci,
                             [[SH * K2 * 4, 128], [K2 * 4, SH], [4, K2]])
                    nc.vector.tensor_copy(dst, cf[:])
                return ccat4

            cbs.append(make_coeffs())

            if hh == 0:
                emit_z_and_wt()

        # ---------- Z (4-corner duplicated padded image) in DRAM ----------
        # ---------- main loop ----------
        out_sb = dpool.tile([128, HW], F32, tag="big16k")
        z_src = AP(z_d.tensor, 0, [[512, GROWS], [1, 512]])
        shp = [128, SH, 128]
        for h in range(NHALF):
            cb4 = cbs[h]
            outp = ppool.tile([128, PIX_PER_HALF], F32, tag="out", name="outp")
            for k in range(K2):
                gZ = gpool.tile([128, SH, 512], BF16, tag="gZ", name="gZ")
                nc.gpsimd.dma_gather(
                    gZ[:], z_src,
                    idxw[:, 1152 * h + 128 * k: 1152 * h + 128 * k + 128],
                    PIX_PER_HALF, PIX_PER_HALF, 512, elem_step=512,
                    single_packet=False,
                )

                def cb(ci):
                    s_ = cb4[:, :, k, ci:ci + 1]
                    return s_.to_broadcast(shp)

                a = vpool.tile(shp, BF16, tag="pa", name="pa")
                nc.vector.tensor_tensor(a[:], gZ[:, :, 0:128], cb(0), Alu.mult)
                b = vpool.tile(shp, BF16, tag="pb", name="pb")
                nc.vector.tensor_tensor(b[:], gZ[:, :, 128:256], cb(1), Alu.mult)
                v = vpool.tile(shp, BF16, tag="pv", name="pv")
                nc.vector.tensor_tensor(v[:], a[:], b[:], Alu.add)
                a2 = vpool.tile(shp, BF16, tag="pa", name="pa")
                nc.vector.tensor_tensor(a2[:], gZ[:, :, 256:384], cb(2), Alu.mult)
                b2 = vpool.tile(shp, BF16, tag="pb", name="pb")
                nc.vector.tensor_tensor(b2[:], gZ[:, :, 384:512], cb(3), Alu.mult)
                v2 = vpool.tile(shp, BF16, tag="pv", name="pv")
                nc.vector.tensor_tensor(v2[:], a2[:], b2[:], Alu.add)
                nc.vector.tensor_tensor(v[:], v[:], v2[:], Alu.add)

                vT = vpool.tile([128, 4 * 512], BF16, tag="vT", name="vT")
                for t in range(SH):
                    trp = tpool.tile([128, 128], BF16, tag="trb", name="trp")
                    nc.tensor.transpose(trp[:], v[:, t, :], ident_bf)
                    nc.scalar.copy(vT[:, t * 128:(t + 1) * 128], trp[:])
                for bk in range(4):
                    nc.tensor.matmul(
                        outp[:, bk * 512:(bk + 1) * 512], wT[:, k, :],
                        vT[:, bk * 512:(bk + 1) * 512],
                        start=(k == 0), stop=(k == K2 - 1),
                    )
            for bk in range(4):
                nc.scalar.activation(
                    out_sb[:, h * PIX_PER_HALF + bk * 512:
                           h * PIX_PER_HALF + (bk + 1) * 512],
                    outp[:, bk * 512:(bk + 1) * 512],
                    mybir.ActivationFunctionType.Identity, bias=bias_sb[:],
                    scale=1.0,
                )
        for q in range(4):
            nc.sync.dma_start(out_d[:, q * 1024:(q + 1) * 1024],
                              out_sb[:, q * 1024:(q + 1) * 1024])


def _make_consts():
    c = np.zeros((128, 707), np.float32)
    c[:, 0:128] = np.eye(128, dtype=np.float32)
    p = np.arange(128)
    c[:, 128] = p
    c[:, 129] = (p >= 64)
    c[:, 130] = p % 64
    s = np.arange(32)[:, None, None]
    kyv = np.arange(3)[None, :, None]
    kxv = np.arange(3)[None, None, :]
    c[:, 131:419] = np.broadcast_to(
        (2 * s + kyv - 1 + PADG + 0 * kxv).reshape(-1), (128, 288))
    c[:, 419:707] = np.broadcast_to(
        (0 * s + 0 * kyv + kxv - 1 + PADG).reshape(-1), (128, 288))
    return c


_COMPILED = None


def _get_compiled():
    global _COMPILED
    if _COMPILED is None:
        nc = bacc.Bacc(get_trn_type() or "TRN2", target_bir_lowering=False,
                       debug=False, num_devices=B)
        with tile.TileContext(nc) as tc:
            _emit(tc)
        nc.compile()
        _COMPILED = nc
    return _COMPILED


def kernel(x, w_om, b_om, weight, bias):
    global LAST_EXEC_TIME_NS
    x = np.ascontiguousarray(np.asarray(x, dtype=np.float32))
    w_om_f = np.ascontiguousarray(np.asarray(w_om, np.float32).reshape(27, 1152))
    b_om_f = np.ascontiguousarray(np.asarray(b_om, np.float32).reshape(27, 1))
    weight_f = np.ascontiguousarray(np.asarray(weight, np.float32).reshape(128, 1152))
    bias_f = np.ascontiguousarray(np.asarray(bias, np.float32).reshape(128, 1))

    nc = _get_compiled()
    consts = _make_consts()
    in_maps = [
        {
            "x": np.ascontiguousarray(x[b].reshape(C, HW)),
            "w_om": w_om_f,
            "b_om": b_om_f,
            "weight": weight_f,
            "bias": bias_f,
            "consts": consts,
        }
        for b in range(B)
    ]
    trace = bool(os.environ.get("DCN_TRACE"))
    res = run_bass_kernel_spmd(nc, in_maps, core_ids=list(range(B)), trace=trace)
    LAST_EXEC_TIME_NS = res.exec_time_ns
    out = np.stack([res.results[b]["out"].reshape(C, H, W) for b in range(B)])
    return out.astype(np.float32)
